# revision 24
# baseline (speedup 1.0000x reference)
"""Trainium2 Bass kernel for CyclicShiftConv (Hilbert-rotation SE attention).

out[b,c,l] = sum_r softmax_r(MLP(mean_l x[b,c,rot_idx[r,l]]))[b,c,r] * x[b,c,rot_idx[r,l]]

Key mathematical facts exploited (verified at runtime in _derive_structure):
  1. Every rot_idx[r] is a PERMUTATION of [0, L).  Hence
     mean_l x[b,c,rot_idx[r,l]] is the same value for every r, so the MLP
     scores are identical across rotations and the softmax weights are
     exactly 1/4.  The whole SE-MLP collapses:
         out = 0.25 * (x + x_rot90 + x_rot180 + x_rot270).
  2. The Hilbert-curve rotation permutations have perfect block structure:
     every aligned 64-block of destination indices gathers from exactly one
     aligned 64-block of source indices, with only ~6 distinct intra-block
     patterns (12 distinct (pattern, 64-parity) pairs).  So each permutation
     is a PE matmul against a small set of constant one-hot routing matrices
     (entries 0.25 to fold in the softmax weight):
         psum[bc, j*64:(j+1)*64] += xT[s128-block]^T @ RM[pattern]
     This replaces the baseline's 32 MiB/core of DMA gather traffic with
     ~20us of Tensor-engine time.

Strategy (8 cores, data-parallel over batch; 2 samples = 512 (b,c) rows/core):
  - load x as bf16 (host converts; tolerance is 2e-2, bf16 adds ~2.4e-3)
  - PE-transpose x -> xT in SBUF (needed as matmul stationary)
  - 768 routing matmuls (64 moving cols each) accumulate the three rotated
    images, pre-scaled by 0.25, into PSUM
  - one fused scalar_tensor_tensor per (wave, bc-tile):
        out = (x * 0.25) + psum
  - DMA out as bf16; host upcasts to f32.
"""

import sys

for _p in ("/opt/trn_rl_repo", "/opt/pypackages"):
    if _p not in sys.path:
        sys.path.append(_p)

import numpy as np

B, C, L = 16, 256, 4096
R = 4
NCORES = 8
BPC = B // NCORES          # samples per core
BC = BPC * C               # 512 rows per core
NT4 = BC // 128            # 4 bc tiles
NB64 = L // 64             # 64 dst 64-blocks
NKILO = 4                  # 1024-col kilo blocks
NW = 8                     # 512-col dst waves
NKMAX = 12                 # max distinct routing matrices

_NC_CACHE = {}


def _derive_structure(rot_idx):
    """Derive the routing structure from the actual rot_idx at runtime."""
    rot = np.asarray(rot_idx, np.int64)
    assert rot.shape == (R, L)
    for r in range(R):
        assert np.array_equal(np.sort(rot[r]), np.arange(L)), (
            "rot_idx rows must be permutations (softmax-collapse precondition)")
    assert np.array_equal(rot[0], np.arange(L)), "rotation 0 must be identity"

    pats = {}
    KIDX = np.zeros((R, NB64), np.int64)
    SRC128 = np.zeros((R, NB64), np.int64)
    for r in (1, 2, 3):
        for j in range(NB64):
            src = rot[r, j * 64:(j + 1) * 64]
            m = int(src[0]) // 64
            assert np.all(src // 64 == m), "64-block structure violated"
            key = (tuple((src % 64).tolist()), m % 2)
            KIDX[r, j] = pats.setdefault(key, len(pats))
            SRC128[r, j] = m // 2
    assert len(pats) <= NKMAX, f"too many routing patterns: {len(pats)}"

    RM = np.zeros((128, NKMAX * 64), np.float32)
    for (pi, parity), k in pats.items():
        RM[np.asarray(pi, np.int64) + parity * 64, k * 64 + np.arange(64)] = 0.25

    # source kilo-blocks needed by each 512-col dst wave
    need = []
    for w in range(NW):
        js = range(w * 8, (w + 1) * 8)
        need.append({int(SRC128[r, j]) // 8 for r in (1, 2, 3) for j in js})
    # load last the kilo that the most waves can do without
    best = max(range(NKILO), key=lambda k: sum(k not in s for s in need))
    LO = [k for k in range(NKILO) if k != best] + [best]
    early = [w for w in range(NW) if best not in need[w]]
    late = [w for w in range(NW) if best in need[w]]
    return RM, KIDX, SRC128, LO, early, late


def _build_nc(KIDX, SRC128, LO, early, late):
    import concourse.mybir as mybir
    from concourse import bacc
    from concourse.tile import TileContext
    from contextlib import ExitStack

    f32 = mybir.dt.float32
    bf16 = mybir.dt.bfloat16
    ALU = mybir.AluOpType
    CSTW = NKMAX * 64 + 128

    nc = bacc.Bacc(
        "TRN2",
        target_bir_lowering=False,
        debug=False,
        enable_asserts=False,
        num_devices=NCORES,
    )

    x_in = nc.dram_tensor("x", [BC, L], bf16, kind="ExternalInput").ap()
    cst_in = nc.dram_tensor("cst", [128, CSTW], bf16, kind="ExternalInput").ap()
    out = nc.dram_tensor("out", [BC, L], bf16, kind="ExternalOutput").ap()

    with TileContext(nc) as tc, ExitStack() as ctx:
        cpool = ctx.enter_context(tc.tile_pool(name="consts", bufs=1))
        xpool = ctx.enter_context(tc.tile_pool(name="xs", bufs=1))
        tpool = ctx.enter_context(tc.tile_pool(name="xT", bufs=1))
        opool = ctx.enter_context(tc.tile_pool(name="ostage", bufs=1))

        cst = cpool.tile([128, CSTW], bf16, name="cst")
        nc.sync.dma_start(cst[:], cst_in)
        rm = cst[:, 0:NKMAX * 64]
        ident = cst[:, NKMAX * 64:NKMAX * 64 + 128]

        xs = [xpool.tile([128, L], bf16, name=f"xs{t}") for t in range(NT4)]
        xT = [tpool.tile([128, 8, BC], bf16, name=f"xT{k}") for k in range(NKILO)]
        ost = [opool.tile([128, L], bf16, name=f"os{t}") for t in range(NT4)]
        gpool = ctx.enter_context(tc.tile_pool(name="gtmp", bufs=3))

        # all input loads up front, in kilo load-order (transfers serialize
        # on the DMA engines in issue order)
        for k in LO:
            for t in range(NT4):
                nc.sync.dma_start(
                    xs[t][:, k * 1024:(k + 1) * 1024],
                    x_in[t * 128:(t + 1) * 128, k * 1024:(k + 1) * 1024],
                )

        # xq = 0.25 * x, precomputed with cheap 4x-mode tensor_scalar ops on
        # the otherwise-idle DVE during the load phase; finals then become
        # ost = xq + psum (tensor_tensor add) or a fused stt
        xq = [opool.tile([128, L], bf16, name=f"xq{t}") for t in range(NT4)]
        for ki, k in enumerate(LO):
            for t in range(NT4):
                # split between idle Pool and fast DVE to keep DVE free for
                # the psum evictions during the load window
                eng = nc.gpsimd if (ki * NT4 + t) % 2 == 0 else nc.vector
                eng.tensor_scalar_mul(
                    xq[t][:, k * 1024:(k + 1) * 1024],
                    xs[t][:, k * 1024:(k + 1) * 1024], 0.25)

        state = {"ev": 0, "fin": 0}

        with (
            tc.tile_pool(name="pb", bufs=2, space="PSUM") as pbpool,
            tc.tile_pool(name="pc", bufs=6, space="PSUM") as pcpool,
        ):
            def do_kilo(k):
                # transpose kilo k of x into xT[k], two l-blocks per psum
                # tile ([128,1024] bf16 = one 2KB bank), evicted in one op
                for e2 in range(4):
                    pb = pbpool.tile([128, 2, BC], bf16, name="pb")
                    for eh in range(2):
                        lb = k * 8 + e2 * 2 + eh
                        for t in range(NT4):
                            nc.tensor.transpose(
                                pb[:, eh, t * 128:(t + 1) * 128],
                                xs[t][:, lb * 128:(lb + 1) * 128],
                                ident,
                            )
                    # DVE evicts run in 2x mode (392ns/512 cols); ACT takes
                    # every third one to keep DVE free for xq work
                    i = state["ev"] % 3
                    state["ev"] += 1
                    dst = xT[k][:, e2 * 2:e2 * 2 + 2, :]
                    if i == 2:
                        nc.scalar.copy(dst, pb[:])
                    else:
                        nc.vector.tensor_copy(dst, pb[:])

            pcs = {}       # (w, t) -> psum tile
            started = {}   # (w, t) -> per-jj contribution count

            def wave_matmuls(w, t, kilos):
                # emit the matmuls of group (w, t) whose source kilo-block is
                # in `kilos`; contributions accumulate into 64-col psum
                # slices with start on first / stop on third arrival
                if (w, t) not in pcs:
                    pcs[(w, t)] = pcpool.tile([128, 512], f32, name="pc")
                    started[(w, t)] = [0] * 8
                pc = pcs[(w, t)]
                cnt = started[(w, t)]
                for jj in range(8):
                    j = w * 8 + jj
                    for r in (1, 2, 3):
                        s = int(SRC128[r, j])
                        if s // 8 not in kilos:
                            continue
                        kk = int(KIDX[r, j])
                        nc.tensor.matmul(
                            pc[:, jj * 64:(jj + 1) * 64],
                            xT[s // 8][:, s % 8, t * 128:(t + 1) * 128],
                            rm[:, kk * 64:(kk + 1) * 64],
                            start=(cnt[jj] == 0),
                            stop=(cnt[jj] == 2),
                        )
                        cnt[jj] += 1

            def wave_final(w, t, fast=False):
                pc = pcs.pop((w, t))
                assert all(c == 3 for c in started.pop((w, t)))
                c0 = w * 512
                osl = ost[t][:, c0:c0 + 512]
                # GPSIMD cannot access PSUM (or run TensorScalarPtr):
                # rotate DVE-solo stt | ACT evict + Pool tt-add |
                # ACT evict + DVE tt-add; `fast` (used on the final wave)
                # avoids the slow Pool path so the tail drains quickly
                m = state["fin"] % 4
                if fast and m == 1:
                    m = 3
                if m in (0, 2):
                    nc.vector.scalar_tensor_tensor(
                        osl, xs[t][:, c0:c0 + 512],
                        0.25, pc[:], ALU.mult, ALU.add)
                else:
                    gt = gpool.tile([128, 512], bf16, name="gt")
                    nc.scalar.copy(gt[:], pc[:])
                    if m == 1:
                        nc.gpsimd.tensor_tensor(
                            osl, xq[t][:, c0:c0 + 512], gt[:], op=ALU.add)
                    else:
                        nc.vector.tensor_tensor(
                            osl, xq[t][:, c0:c0 + 512], gt[:], op=ALU.add)
                state["fin"] += 1
                # stream the result out immediately; alternate the issuing
                # sequencer (SP/ACT) so the tail is not issue-rate-bound
                deng = nc.sync if state["fin"] % 2 == 0 else nc.scalar
                deng.dma_start(
                    out[t * 128:(t + 1) * 128, c0:c0 + 512], osl)

            ALLK = set(range(NKILO))
            w0, w1 = early[0], early[1]
            do_kilo(LO[0])
            do_kilo(LO[1])
            do_kilo(LO[2])
            for t in range(NT4):
                wave_matmuls(w0, t, ALLK)
                wave_final(w0, t)
            for t in range(NT4):
                wave_matmuls(w1, t, ALLK)
                wave_final(w1, t)
            do_kilo(LO[3])
            for wi, w in enumerate(late):
                last = wi == len(late) - 1
                for t in range(NT4):
                    wave_matmuls(w, t, ALLK)
                    wave_final(w, t, fast=last)

    nc.compile()
    return nc


def _host_prep(x, rot_idx):
    import ml_dtypes

    bf = ml_dtypes.bfloat16
    RM = _NC_CACHE["RM"]
    cst = np.zeros((128, NKMAX * 64 + 128), np.float32)
    cst[:, :NKMAX * 64] = RM
    cst[:, NKMAX * 64:] = np.eye(128, dtype=np.float32)
    cst = cst.astype(bf)

    x = np.asarray(x, dtype=np.float32)
    in_maps = []
    for c in range(NCORES):
        xc = np.ascontiguousarray(
            x[c * BPC:(c + 1) * BPC].reshape(BC, L)).astype(bf)
        in_maps.append({"x": xc, "cst": cst})
    return in_maps


def kernel(x, rot_idx, w1, b1, w2, b2, _trace=False):
    # w1/b1/w2/b2 provably do not affect the output when every rot_idx row
    # is a permutation (asserted in _derive_structure): the SE-MLP sees the
    # same mean for every rotation, so the softmax is uniform.
    from concourse import bass_utils

    key = np.asarray(rot_idx, np.int32).tobytes()
    if _NC_CACHE.get("key") != key:
        RM, KIDX, SRC128, LO, early, late = _derive_structure(rot_idx)
        _NC_CACHE["RM"] = RM
        _NC_CACHE["nc"] = _build_nc(KIDX, SRC128, LO, early, late)
        _NC_CACHE["key"] = key
    nc = _NC_CACHE["nc"]

    in_maps = _host_prep(x, rot_idx)
    res = bass_utils.run_bass_kernel_spmd(
        nc, in_maps, core_ids=list(range(NCORES)), trace=_trace
    )
    out = np.empty((B, C, L), dtype=np.float32)
    for c in range(NCORES):
        out[c * BPC:(c + 1) * BPC] = (
            res.results[c]["out"].astype(np.float32).reshape(BPC, C, L))
    if _trace:
        kernel.last_results = res
    return out


# revision 25
# speedup vs baseline: 1.0111x; 1.0111x over previous
"""Trainium2 Bass kernel for CyclicShiftConv (Hilbert-rotation SE attention).

out[b,c,l] = sum_r softmax_r(MLP(mean_l x[b,c,rot_idx[r,l]]))[b,c,r] * x[b,c,rot_idx[r,l]]

Key mathematical facts exploited (verified at runtime in _derive_structure):
  1. Every rot_idx[r] is a PERMUTATION of [0, L).  Hence
     mean_l x[b,c,rot_idx[r,l]] is the same value for every r, so the MLP
     scores are identical across rotations and the softmax weights are
     exactly 1/4.  The whole SE-MLP collapses:
         out = 0.25 * (x + x_rot90 + x_rot180 + x_rot270).
  2. The Hilbert-curve rotation permutations have perfect block structure:
     every aligned 64-block of destination indices gathers from exactly one
     aligned 64-block of source indices, with only ~6 distinct intra-block
     patterns (12 distinct (pattern, 64-parity) pairs).  So each permutation
     is a PE matmul against a small set of constant one-hot routing matrices
     (entries 0.25 to fold in the softmax weight):
         psum[bc, j*64:(j+1)*64] += xT[s128-block]^T @ RM[pattern]
     This replaces the baseline's 32 MiB/core of DMA gather traffic with
     ~20us of Tensor-engine time.

Strategy (8 cores, data-parallel over batch; 2 samples = 512 (b,c) rows/core):
  - load x as bf16 (host converts; tolerance is 2e-2, bf16 adds ~2.4e-3)
  - PE-transpose x -> xT in SBUF (needed as matmul stationary)
  - 768 routing matmuls (64 moving cols each) accumulate the three rotated
    images, pre-scaled by 0.25, into PSUM
  - one fused scalar_tensor_tensor per (wave, bc-tile):
        out = (x * 0.25) + psum
  - DMA out as bf16; host upcasts to f32.
"""

import sys

for _p in ("/opt/trn_rl_repo", "/opt/pypackages"):
    if _p not in sys.path:
        sys.path.append(_p)

import numpy as np

B, C, L = 16, 256, 4096
R = 4
NCORES = 8
BPC = B // NCORES          # samples per core
BC = BPC * C               # 512 rows per core
NT4 = BC // 128            # 4 bc tiles
NB64 = L // 64             # 64 dst 64-blocks
NKILO = 4                  # 1024-col kilo blocks
NW = 8                     # 512-col dst waves
NKMAX = 12                 # max distinct routing matrices

_NC_CACHE = {}


def _derive_structure(rot_idx):
    """Derive the routing structure from the actual rot_idx at runtime."""
    rot = np.asarray(rot_idx, np.int64)
    assert rot.shape == (R, L)
    for r in range(R):
        assert np.array_equal(np.sort(rot[r]), np.arange(L)), (
            "rot_idx rows must be permutations (softmax-collapse precondition)")
    assert np.array_equal(rot[0], np.arange(L)), "rotation 0 must be identity"

    pats = {}
    KIDX = np.zeros((R, NB64), np.int64)
    SRC128 = np.zeros((R, NB64), np.int64)
    for r in (1, 2, 3):
        for j in range(NB64):
            src = rot[r, j * 64:(j + 1) * 64]
            m = int(src[0]) // 64
            assert np.all(src // 64 == m), "64-block structure violated"
            key = (tuple((src % 64).tolist()), m % 2)
            KIDX[r, j] = pats.setdefault(key, len(pats))
            SRC128[r, j] = m // 2
    assert len(pats) <= NKMAX, f"too many routing patterns: {len(pats)}"

    RM = np.zeros((128, NKMAX * 64), np.float32)
    for (pi, parity), k in pats.items():
        RM[np.asarray(pi, np.int64) + parity * 64, k * 64 + np.arange(64)] = 0.25

    # source kilo-blocks needed by each 512-col dst wave
    need = []
    for w in range(NW):
        js = range(w * 8, (w + 1) * 8)
        need.append({int(SRC128[r, j]) // 8 for r in (1, 2, 3) for j in js})
    # load last the kilo that the most waves can do without
    best = max(range(NKILO), key=lambda k: sum(k not in s for s in need))
    LO = [k for k in range(NKILO) if k != best] + [best]
    early = [w for w in range(NW) if best not in need[w]]
    late = [w for w in range(NW) if best in need[w]]
    return RM, KIDX, SRC128, LO, early, late


def _build_nc(KIDX, SRC128, LO, early, late):
    import concourse.mybir as mybir
    from concourse import bacc
    from concourse.tile import TileContext
    from contextlib import ExitStack

    f32 = mybir.dt.float32
    bf16 = mybir.dt.bfloat16
    ALU = mybir.AluOpType
    CSTW = NKMAX * 64 + 128

    nc = bacc.Bacc(
        "TRN2",
        target_bir_lowering=False,
        debug=False,
        enable_asserts=False,
        num_devices=NCORES,
    )

    x_in = nc.dram_tensor("x", [BC, L], bf16, kind="ExternalInput").ap()
    cst_in = nc.dram_tensor("cst", [128, CSTW], bf16, kind="ExternalInput").ap()
    out = nc.dram_tensor("out", [BC, L], bf16, kind="ExternalOutput").ap()

    with TileContext(nc) as tc, ExitStack() as ctx:
        cpool = ctx.enter_context(tc.tile_pool(name="consts", bufs=1))
        xpool = ctx.enter_context(tc.tile_pool(name="xs", bufs=1))
        tpool = ctx.enter_context(tc.tile_pool(name="xT", bufs=1))
        opool = ctx.enter_context(tc.tile_pool(name="ostage", bufs=1))

        cst = cpool.tile([128, CSTW], bf16, name="cst")
        nc.sync.dma_start(cst[:], cst_in)
        rm = cst[:, 0:NKMAX * 64]
        ident = cst[:, NKMAX * 64:NKMAX * 64 + 128]

        xs = [xpool.tile([128, L], bf16, name=f"xs{t}") for t in range(NT4)]
        xT = [tpool.tile([128, 8, BC], bf16, name=f"xT{k}") for k in range(NKILO)]
        ost = [opool.tile([128, L], bf16, name=f"os{t}") for t in range(NT4)]
        gpool = ctx.enter_context(tc.tile_pool(name="gtmp", bufs=3))

        # all input loads up front, in kilo load-order (transfers serialize
        # on the DMA engines in issue order)
        for k in LO:
            for t in range(NT4):
                nc.sync.dma_start(
                    xs[t][:, k * 1024:(k + 1) * 1024],
                    x_in[t * 128:(t + 1) * 128, k * 1024:(k + 1) * 1024],
                )

        # xq = 0.25 * x, precomputed with cheap 4x-mode tensor_scalar ops on
        # the otherwise-idle DVE during the load phase; finals then become
        # ost = xq + psum (tensor_tensor add) or a fused stt
        xq = [opool.tile([128, L], bf16, name=f"xq{t}") for t in range(NT4)]
        for ki, k in enumerate(LO):
            for t in range(NT4):
                # split between idle Pool and fast DVE to keep DVE free for
                # the psum evictions during the load window
                eng = nc.gpsimd if (ki * NT4 + t) % 2 == 0 else nc.vector
                eng.tensor_scalar_mul(
                    xq[t][:, k * 1024:(k + 1) * 1024],
                    xs[t][:, k * 1024:(k + 1) * 1024], 0.25)

        state = {"ev": 0, "fin": 0}

        with (
            tc.tile_pool(name="pb", bufs=3, space="PSUM") as pbpool,
            tc.tile_pool(name="pc", bufs=5, space="PSUM") as pcpool,
        ):
            def do_kilo(k):
                # transpose kilo k of x into xT[k], two l-blocks per psum
                # tile ([128,1024] bf16 = one 2KB bank), evicted in one op
                for e2 in range(4):
                    pb = pbpool.tile([128, 2, BC], bf16, name="pb")
                    for eh in range(2):
                        lb = k * 8 + e2 * 2 + eh
                        for t in range(NT4):
                            nc.tensor.transpose(
                                pb[:, eh, t * 128:(t + 1) * 128],
                                xs[t][:, lb * 128:(lb + 1) * 128],
                                ident,
                            )
                    # DVE evicts run in 2x mode (392ns/512 cols); ACT takes
                    # every third one to keep DVE free for xq work
                    i = state["ev"] % 3
                    state["ev"] += 1
                    dst = xT[k][:, e2 * 2:e2 * 2 + 2, :]
                    if i == 2:
                        nc.scalar.copy(dst, pb[:])
                    else:
                        nc.vector.tensor_copy(dst, pb[:])

            pcs = {}       # (w, t) -> psum tile
            started = {}   # (w, t) -> per-jj contribution count

            def wave_matmuls(w, t, kilos):
                # emit the matmuls of group (w, t) whose source kilo-block is
                # in `kilos`; contributions accumulate into 64-col psum
                # slices with start on first / stop on third arrival
                if (w, t) not in pcs:
                    pcs[(w, t)] = pcpool.tile([128, 512], f32, name="pc")
                    started[(w, t)] = [0] * 8
                pc = pcs[(w, t)]
                cnt = started[(w, t)]
                for jj in range(8):
                    j = w * 8 + jj
                    for r in (1, 2, 3):
                        s = int(SRC128[r, j])
                        if s // 8 not in kilos:
                            continue
                        kk = int(KIDX[r, j])
                        nc.tensor.matmul(
                            pc[:, jj * 64:(jj + 1) * 64],
                            xT[s // 8][:, s % 8, t * 128:(t + 1) * 128],
                            rm[:, kk * 64:(kk + 1) * 64],
                            start=(cnt[jj] == 0),
                            stop=(cnt[jj] == 2),
                        )
                        cnt[jj] += 1

            def wave_final(w, t, fast=False):
                pc = pcs.pop((w, t))
                assert all(c == 3 for c in started.pop((w, t)))
                c0 = w * 512
                osl = ost[t][:, c0:c0 + 512]
                # GPSIMD cannot access PSUM (or run TensorScalarPtr):
                # rotate DVE-solo stt | ACT evict + Pool tt-add |
                # ACT evict + DVE tt-add; `fast` (used on the final wave)
                # avoids the slow Pool path so the tail drains quickly
                m = state["fin"] % 4
                if fast and m == 1:
                    m = 3
                if m in (0, 2):
                    nc.vector.scalar_tensor_tensor(
                        osl, xs[t][:, c0:c0 + 512],
                        0.25, pc[:], ALU.mult, ALU.add)
                else:
                    gt = gpool.tile([128, 512], bf16, name="gt")
                    nc.scalar.copy(gt[:], pc[:])
                    if m == 1:
                        nc.gpsimd.tensor_tensor(
                            osl, xq[t][:, c0:c0 + 512], gt[:], op=ALU.add)
                    else:
                        nc.vector.tensor_tensor(
                            osl, xq[t][:, c0:c0 + 512], gt[:], op=ALU.add)
                state["fin"] += 1
                # stream the result out immediately; alternate the issuing
                # sequencer (SP/ACT) so the tail is not issue-rate-bound
                deng = nc.sync if state["fin"] % 2 == 0 else nc.scalar
                deng.dma_start(
                    out[t * 128:(t + 1) * 128, c0:c0 + 512], osl)

            ALLK = set(range(NKILO))
            w0, w1 = early[0], early[1]
            do_kilo(LO[0])
            do_kilo(LO[1])
            do_kilo(LO[2])
            for t in range(NT4):
                wave_matmuls(w0, t, ALLK)
                wave_final(w0, t)
            for t in range(NT4):
                wave_matmuls(w1, t, ALLK)
                wave_final(w1, t)
            do_kilo(LO[3])
            for wi, w in enumerate(late):
                last = wi == len(late) - 1
                for t in range(NT4):
                    wave_matmuls(w, t, ALLK)
                    wave_final(w, t, fast=last)

    nc.compile()
    return nc


def _host_prep(x, rot_idx):
    import ml_dtypes

    bf = ml_dtypes.bfloat16
    RM = _NC_CACHE["RM"]
    cst = np.zeros((128, NKMAX * 64 + 128), np.float32)
    cst[:, :NKMAX * 64] = RM
    cst[:, NKMAX * 64:] = np.eye(128, dtype=np.float32)
    cst = cst.astype(bf)

    x = np.asarray(x, dtype=np.float32)
    in_maps = []
    for c in range(NCORES):
        xc = np.ascontiguousarray(
            x[c * BPC:(c + 1) * BPC].reshape(BC, L)).astype(bf)
        in_maps.append({"x": xc, "cst": cst})
    return in_maps


def kernel(x, rot_idx, w1, b1, w2, b2, _trace=False):
    # w1/b1/w2/b2 provably do not affect the output when every rot_idx row
    # is a permutation (asserted in _derive_structure): the SE-MLP sees the
    # same mean for every rotation, so the softmax is uniform.
    from concourse import bass_utils

    key = np.asarray(rot_idx, np.int32).tobytes()
    if _NC_CACHE.get("key") != key:
        RM, KIDX, SRC128, LO, early, late = _derive_structure(rot_idx)
        _NC_CACHE["RM"] = RM
        _NC_CACHE["nc"] = _build_nc(KIDX, SRC128, LO, early, late)
        _NC_CACHE["key"] = key
    nc = _NC_CACHE["nc"]

    in_maps = _host_prep(x, rot_idx)
    res = bass_utils.run_bass_kernel_spmd(
        nc, in_maps, core_ids=list(range(NCORES)), trace=_trace
    )
    out = np.empty((B, C, L), dtype=np.float32)
    for c in range(NCORES):
        out[c * BPC:(c + 1) * BPC] = (
            res.results[c]["out"].astype(np.float32).reshape(BPC, C, L))
    if _trace:
        kernel.last_results = res
    return out


# revision 26
# speedup vs baseline: 1.0289x; 1.0176x over previous
"""Trainium2 Bass kernel for CyclicShiftConv (Hilbert-rotation SE attention).

out[b,c,l] = sum_r softmax_r(MLP(mean_l x[b,c,rot_idx[r,l]]))[b,c,r] * x[b,c,rot_idx[r,l]]

Key mathematical facts exploited (verified at runtime in _derive_structure):
  1. Every rot_idx[r] is a PERMUTATION of [0, L).  Hence
     mean_l x[b,c,rot_idx[r,l]] is the same value for every r, so the MLP
     scores are identical across rotations and the softmax weights are
     exactly 1/4.  The whole SE-MLP collapses:
         out = 0.25 * (x + x_rot90 + x_rot180 + x_rot270).
  2. The Hilbert-curve rotation permutations have perfect block structure:
     every aligned 64-block of destination indices gathers from exactly one
     aligned 64-block of source indices, with only ~6 distinct intra-block
     patterns (12 distinct (pattern, 64-parity) pairs).  So each permutation
     is a PE matmul against a small set of constant one-hot routing matrices
     (entries 0.25 to fold in the softmax weight):
         psum[bc, j*64:(j+1)*64] += xT[s128-block]^T @ RM[pattern]
     This replaces the baseline's 32 MiB/core of DMA gather traffic with
     ~20us of Tensor-engine time.

Strategy (8 cores, data-parallel over batch; 2 samples = 512 (b,c) rows/core):
  - load x as bf16 (host converts; tolerance is 2e-2, bf16 adds ~2.4e-3)
  - PE-transpose x -> xT in SBUF (needed as matmul stationary)
  - 768 routing matmuls (64 moving cols each) accumulate the three rotated
    images, pre-scaled by 0.25, into PSUM
  - one fused scalar_tensor_tensor per (wave, bc-tile):
        out = (x * 0.25) + psum
  - DMA out as bf16; host upcasts to f32.
"""

import sys

for _p in ("/opt/trn_rl_repo", "/opt/pypackages"):
    if _p not in sys.path:
        sys.path.append(_p)

import numpy as np

B, C, L = 16, 256, 4096
R = 4
NCORES = 8
BPC = B // NCORES          # samples per core
BC = BPC * C               # 512 rows per core
NT4 = BC // 128            # 4 bc tiles
NB64 = L // 64             # 64 dst 64-blocks
NKILO = 4                  # 1024-col kilo blocks
NW = 8                     # 512-col dst waves
NKMAX = 12                 # max distinct routing matrices

_NC_CACHE = {}


def _derive_structure(rot_idx):
    """Derive the routing structure from the actual rot_idx at runtime."""
    rot = np.asarray(rot_idx, np.int64)
    assert rot.shape == (R, L)
    for r in range(R):
        assert np.array_equal(np.sort(rot[r]), np.arange(L)), (
            "rot_idx rows must be permutations (softmax-collapse precondition)")
    assert np.array_equal(rot[0], np.arange(L)), "rotation 0 must be identity"

    pats = {}
    KIDX = np.zeros((R, NB64), np.int64)
    SRC128 = np.zeros((R, NB64), np.int64)
    for r in (1, 2, 3):
        for j in range(NB64):
            src = rot[r, j * 64:(j + 1) * 64]
            m = int(src[0]) // 64
            assert np.all(src // 64 == m), "64-block structure violated"
            key = (tuple((src % 64).tolist()), m % 2)
            KIDX[r, j] = pats.setdefault(key, len(pats))
            SRC128[r, j] = m // 2
    assert len(pats) <= NKMAX, f"too many routing patterns: {len(pats)}"

    RM = np.zeros((128, NKMAX * 64), np.float32)
    for (pi, parity), k in pats.items():
        RM[np.asarray(pi, np.int64) + parity * 64, k * 64 + np.arange(64)] = 0.25

    # source kilo-blocks needed by each 512-col dst wave
    need = []
    for w in range(NW):
        js = range(w * 8, (w + 1) * 8)
        need.append({int(SRC128[r, j]) // 8 for r in (1, 2, 3) for j in js})
    # load last the kilo that the most waves can do without
    best = max(range(NKILO), key=lambda k: sum(k not in s for s in need))
    LO = [k for k in range(NKILO) if k != best] + [best]
    early = [w for w in range(NW) if best not in need[w]]
    late = [w for w in range(NW) if best in need[w]]
    return RM, KIDX, SRC128, LO, early, late


def _build_nc(KIDX, SRC128, LO, early, late):
    import concourse.mybir as mybir
    from concourse import bacc
    from concourse.tile import TileContext
    from contextlib import ExitStack

    f32 = mybir.dt.float32
    bf16 = mybir.dt.bfloat16
    ALU = mybir.AluOpType
    CSTW = NKMAX * 64 + 128

    nc = bacc.Bacc(
        "TRN2",
        target_bir_lowering=False,
        debug=False,
        enable_asserts=False,
        num_devices=NCORES,
    )

    x_in = nc.dram_tensor("x", [BC, L], bf16, kind="ExternalInput").ap()
    cst_in = nc.dram_tensor("cst", [128, CSTW], bf16, kind="ExternalInput").ap()
    out = nc.dram_tensor("out", [BC, L], bf16, kind="ExternalOutput").ap()

    with TileContext(nc) as tc, ExitStack() as ctx:
        cpool = ctx.enter_context(tc.tile_pool(name="consts", bufs=1))
        xpool = ctx.enter_context(tc.tile_pool(name="xs", bufs=1))
        tpool = ctx.enter_context(tc.tile_pool(name="xT", bufs=1))
        opool = ctx.enter_context(tc.tile_pool(name="ostage", bufs=1))

        cst = cpool.tile([128, CSTW], bf16, name="cst")
        nc.sync.dma_start(cst[:], cst_in)
        rm = cst[:, 0:NKMAX * 64]
        ident = cst[:, NKMAX * 64:NKMAX * 64 + 128]

        xs = [xpool.tile([128, L], bf16, name=f"xs{t}") for t in range(NT4)]
        xT = [tpool.tile([128, 8, BC], bf16, name=f"xT{k}") for k in range(NKILO)]
        ost = [opool.tile([128, L], bf16, name=f"os{t}") for t in range(NT4)]
        gpool = ctx.enter_context(tc.tile_pool(name="gtmp", bufs=3))

        # all input loads up front, in kilo load-order (transfers serialize
        # on the DMA engines in issue order)
        for k in LO:
            for t in range(NT4):
                nc.sync.dma_start(
                    xs[t][:, k * 1024:(k + 1) * 1024],
                    x_in[t * 128:(t + 1) * 128, k * 1024:(k + 1) * 1024],
                )

        # xq = 0.25 * x, precomputed with cheap 4x-mode tensor_scalar ops on
        # the otherwise-idle DVE during the load phase; finals then become
        # ost = xq + psum (tensor_tensor add) or a fused stt
        xq = [opool.tile([128, L], bf16, name=f"xq{t}") for t in range(NT4)]
        for ki, k in enumerate(LO):
            for t in range(NT4):
                # split between idle Pool and fast DVE to keep DVE free for
                # the psum evictions during the load window
                eng = nc.gpsimd if (ki * NT4 + t) % 2 == 0 else nc.vector
                eng.tensor_scalar_mul(
                    xq[t][:, k * 1024:(k + 1) * 1024],
                    xs[t][:, k * 1024:(k + 1) * 1024], 0.25)

        state = {"ev": 0, "fin": 0}

        with (
            tc.tile_pool(name="pb", bufs=3, space="PSUM") as pbpool,
            tc.tile_pool(name="pc", bufs=5, space="PSUM") as pcpool,
        ):
            def do_kilo(k):
                # transpose kilo k of x into xT[k], two l-blocks per psum
                # tile ([128,1024] bf16 = one 2KB bank), evicted in one op
                for e2 in range(4):
                    pb = pbpool.tile([128, 2, BC], bf16, name="pb")
                    for eh in range(2):
                        lb = k * 8 + e2 * 2 + eh
                        for t in range(NT4):
                            nc.tensor.transpose(
                                pb[:, eh, t * 128:(t + 1) * 128],
                                xs[t][:, lb * 128:(lb + 1) * 128],
                                ident,
                            )
                    # DVE evicts run in 2x mode (392ns/512 cols); ACT takes
                    # every third one to keep DVE free for xq work
                    i = state["ev"] % 3
                    state["ev"] += 1
                    dst = xT[k][:, e2 * 2:e2 * 2 + 2, :]
                    if i == 2:
                        nc.scalar.copy(dst, pb[:])
                    else:
                        nc.vector.tensor_copy(dst, pb[:])

            pcs = {}       # (w, t) -> psum tile
            started = {}   # (w, t) -> per-jj contribution count

            def wave_matmuls(w, t, kilos):
                # emit the matmuls of group (w, t) whose source kilo-block is
                # in `kilos`; contributions accumulate into 64-col psum
                # slices with start on first / stop on third arrival
                if (w, t) not in pcs:
                    pcs[(w, t)] = pcpool.tile([128, 512], f32, name="pc")
                    started[(w, t)] = [0] * 8
                pc = pcs[(w, t)]
                cnt = started[(w, t)]
                for jj in range(8):
                    j = w * 8 + jj
                    for r in (1, 2, 3):
                        s = int(SRC128[r, j])
                        if s // 8 not in kilos:
                            continue
                        kk = int(KIDX[r, j])
                        nc.tensor.matmul(
                            pc[:, jj * 64:(jj + 1) * 64],
                            xT[s // 8][:, s % 8, t * 128:(t + 1) * 128],
                            rm[:, kk * 64:(kk + 1) * 64],
                            start=(cnt[jj] == 0),
                            stop=(cnt[jj] == 2),
                        )
                        cnt[jj] += 1

            def wave_final(w, t, fast=False):
                pc = pcs.pop((w, t))
                assert all(c == 3 for c in started.pop((w, t)))
                c0 = w * 512
                osl = ost[t][:, c0:c0 + 512]
                # GPSIMD cannot access PSUM (or run TensorScalarPtr):
                # rotate DVE-solo stt | ACT evict + Pool tt-add |
                # ACT evict + DVE tt-add; `fast` (used on the final wave)
                # avoids the slow Pool path so the tail drains quickly
                m = state["fin"] % 4
                if fast and m == 1:
                    m = 3
                if m in (0, 2):
                    nc.vector.scalar_tensor_tensor(
                        osl, xs[t][:, c0:c0 + 512],
                        0.25, pc[:], ALU.mult, ALU.add)
                else:
                    gt = gpool.tile([128, 512], bf16, name="gt")
                    nc.scalar.copy(gt[:], pc[:])
                    if m == 1:
                        nc.gpsimd.tensor_tensor(
                            osl, xq[t][:, c0:c0 + 512], gt[:], op=ALU.add)
                    else:
                        nc.vector.tensor_tensor(
                            osl, xq[t][:, c0:c0 + 512], gt[:], op=ALU.add)
                state["fin"] += 1
                # stream the result out immediately; alternate the issuing
                # sequencer (SP/ACT) so the tail is not issue-rate-bound
                deng = nc.sync if state["fin"] % 2 == 0 else nc.scalar
                deng.dma_start(
                    out[t * 128:(t + 1) * 128, c0:c0 + 512], osl)

            ALLK = set(range(NKILO))
            w0, w1 = early[0], early[1]
            do_kilo(LO[0])
            do_kilo(LO[1])
            do_kilo(LO[2])
            for t in range(NT4):
                wave_matmuls(w0, t, ALLK)
                wave_final(w0, t)
            # transpose the last kilo before w1's finals so its evictions
            # (which gate all six late waves) are early in the DVE/ACT queues
            do_kilo(LO[3])
            for t in range(NT4):
                wave_matmuls(w1, t, ALLK)
                wave_final(w1, t)
            for wi, w in enumerate(late):
                last = wi == len(late) - 1
                for t in range(NT4):
                    wave_matmuls(w, t, ALLK)
                    wave_final(w, t, fast=last)

    nc.compile()
    return nc


def _host_prep(x, rot_idx):
    import ml_dtypes

    bf = ml_dtypes.bfloat16
    RM = _NC_CACHE["RM"]
    cst = np.zeros((128, NKMAX * 64 + 128), np.float32)
    cst[:, :NKMAX * 64] = RM
    cst[:, NKMAX * 64:] = np.eye(128, dtype=np.float32)
    cst = cst.astype(bf)

    x = np.asarray(x, dtype=np.float32)
    in_maps = []
    for c in range(NCORES):
        xc = np.ascontiguousarray(
            x[c * BPC:(c + 1) * BPC].reshape(BC, L)).astype(bf)
        in_maps.append({"x": xc, "cst": cst})
    return in_maps


def kernel(x, rot_idx, w1, b1, w2, b2, _trace=False):
    # w1/b1/w2/b2 provably do not affect the output when every rot_idx row
    # is a permutation (asserted in _derive_structure): the SE-MLP sees the
    # same mean for every rotation, so the softmax is uniform.
    from concourse import bass_utils

    key = np.asarray(rot_idx, np.int32).tobytes()
    if _NC_CACHE.get("key") != key:
        RM, KIDX, SRC128, LO, early, late = _derive_structure(rot_idx)
        _NC_CACHE["RM"] = RM
        _NC_CACHE["nc"] = _build_nc(KIDX, SRC128, LO, early, late)
        _NC_CACHE["key"] = key
    nc = _NC_CACHE["nc"]

    in_maps = _host_prep(x, rot_idx)
    res = bass_utils.run_bass_kernel_spmd(
        nc, in_maps, core_ids=list(range(NCORES)), trace=_trace
    )
    out = np.empty((B, C, L), dtype=np.float32)
    for c in range(NCORES):
        out[c * BPC:(c + 1) * BPC] = (
            res.results[c]["out"].astype(np.float32).reshape(BPC, C, L))
    if _trace:
        kernel.last_results = res
    return out
